# revision 13
# baseline (speedup 1.0000x reference)
"""Causal multi-head attention on 8 TRN2 NeuronCores — v2 (phase-interleaved).

Problem: B=2, T=2048, C=2048, H=16 heads, D=128 head_dim, fp32 reference.

Sharding (hardcoded): tensor-parallel over heads x4 (4 heads per core),
data-parallel over batch x2.  Core i handles batch i//4, head-group i%4
(heads 4*(i%4) .. 4*(i%4)+3).  Each core computes a *partial* output
[T, C] = (softmax(QK^T/sqrt(D)) V)_heads @ wo_shard^T ; the host sums the
4 TP partials per batch (the row-parallel wo all-reduce, done at unshard).

v2 structure: the attention stream for q-chunk c is scalar-engine (exp)
bound, so the PE bubbles there are packed with independent filler work:
the NEXT chunk's Q/K/V projection chains and the PREVIOUS chunk's WO
groups, interleaved Bresenham-style between attention items.  The
denominator is folded to ONE ones-matmul per (head, chunk) by
accumulating all band partials into the quad-stash tile on the DVE.
"""

import math

import ml_dtypes
import numpy as np

import concourse.bass as bass
import concourse.tile as tile
from concourse import bacc, mybir
from concourse.bass_utils import run_bass_kernel_spmd

B, T, C = 2, 2048, 2048
H, D = 16, 128
HG = 4              # head-groups (TP degree); heads per core = H // HG = 4
NH = H // HG        # heads per core
NT = T // 512       # 512-wide t/q chunks
SCALE = 1.0 / math.sqrt(D)

BF16 = mybir.dt.bfloat16
F32 = mybir.dt.float32

NP_BF16 = ml_dtypes.bfloat16

DT = BF16           # all matmul operands bf16; PSUM accumulation is f32
DT_OUT = BF16       # partial-output DMA dtype (host sums partials in f32)


def _build():
    nc = bacc.Bacc("TRN2", target_bir_lowering=False, debug=False, num_devices=8)

    xt = nc.dram_tensor("xt", [128, 16 * T], DT, kind="ExternalInput")
    wqt = nc.dram_tensor("wqt", [128, 8192], DT, kind="ExternalInput")
    wkt = nc.dram_tensor("wkt", [128, 8192], DT, kind="ExternalInput")
    wvt = nc.dram_tensor("wvt", [128, 8192], DT, kind="ExternalInput")
    wot = nc.dram_tensor("wot", [128, 8192], DT, kind="ExternalInput")
    msk = nc.dram_tensor("msk", [128, 2048], DT, kind="ExternalInput")
    out = nc.dram_tensor("out", [T, C], DT_OUT, kind="ExternalOutput")

    with tile.TileContext(nc) as tc:
        with (
            tc.tile_pool(name="big", bufs=1) as big,
            tc.tile_pool(name="xs", bufs=2) as xs,
            tc.tile_pool(name="work", bufs=2) as work,
            tc.tile_pool(name="ps", bufs=2, space="PSUM") as psum,
        ):
            wq_sb = big.tile([128, 8192], DT)
            wk_sb = big.tile([128, 8192], DT)
            wv_sb = big.tile([128, 8192], DT)
            wo_sb = big.tile([128, 8192], DT)
            msk_sb = big.tile([128, 2048], DT)
            ones_k = big.tile([128, 1], DT)
            warm_sb = big.tile([128, 128], DT)
            nc.gpsimd.memset(ones_k[:], 1.0)
            nc.gpsimd.memset(warm_sb[:], 1.0)

            # Startup DMAs on BOTH hardware DGE queues (Sync + Scalar run
            # independent rings): Sync carries the Q-path (wq, x0, x1),
            # Scalar carries the K/V-path (wk, wv) plus msk/wo, so the
            # DMA-bound startup window is nearly halved.
            x_tiles = [None] * NT
            x_tiles[0] = xs.tile([128, 8192], DT, tag="x", name="x_sb")
            x0_sb = x_tiles[0]
            nc.sync.dma_start(wq_sb[:, 0:2048], wqt[:, 0:2048])
            nc.sync.dma_start(x0_sb[:, 0:2048], xt[:, 0:2048])
            nc.sync.dma_start(x0_sb[:, 2048:4096], xt[:, 2048:4096])
            nc.sync.dma_start(x0_sb[:, 4096:6144], xt[:, 4096:6144])
            nc.sync.dma_start(wq_sb[:, 2048:4096], wqt[:, 2048:4096])
            nc.sync.dma_start(x0_sb[:, 6144:8192], xt[:, 6144:8192])
            nc.sync.dma_start(wk_sb[:, 0:2048], wkt[:, 0:2048])
            nc.sync.dma_start(wk_sb[:, 2048:4096], wkt[:, 2048:4096])
            nc.sync.dma_start(wq_sb[:, 4096:8192], wqt[:, 4096:8192])
            nc.sync.dma_start(msk_sb[:], msk[:])
            nc.sync.dma_start(wk_sb[:, 4096:8192], wkt[:, 4096:8192])
            nc.sync.dma_start(wv_sb[:], wvt[:])

            # Dummy matmul stream during the initial DMA wait: keeps the PE
            # busy through the HAM activity window so the real matmuls start
            # at the un-throttled clock.  More warm matmuls are interleaved
            # into the DMA-paced first Q projection below.
            warm_ps = psum.tile([1, 128], F32, tag="s", name="warm_ps", bufs=3)

            def _warm(n):
                for _ in range(n):
                    nc.tensor.matmul(
                        warm_ps[:], lhsT=ones_k[:], rhs=warm_sb[:],
                        start=True, stop=True,
                    )

            _warm(32)

            kT_sb = big.tile([128, NH * T], DT)     # per head: [d=128, t]
            v_sb = big.tile([128, 16 * 512], DT)    # [t=128, (t_tile, 4h*128)]
            oT_sb = big.tile([128, NH * T], DT)     # per head: [d=128, t]

            qT_tiles = [None] * NT

            # ---- chunk-0 Q + K(h0,h1) projections, DMA-paced with warm
            # fill.  K(h2,h3) and V0 become phase-0 filler units, so the PE
            # does real work instead of stalling on the late wk/wv DMAs.
            qT_tiles[0] = work.tile([128, NH * 512], DT, tag="qT", name="qT")
            for h, kind in [(0, "q"), (1, "q"), (0, "k"), (1, "k"),
                            (2, "q"), (3, "q")]:
                w_sb0 = wq_sb if kind == "q" else wk_sb
                ps = psum.tile([128, 512], F32, tag="acc", bufs=2)
                for ci in range(16):
                    nc.tensor.matmul(
                        ps[:],
                        lhsT=w_sb0[:, 2048 * h + 128 * ci:2048 * h + 128 * (ci + 1)],
                        rhs=x0_sb[:, 512 * ci:512 * (ci + 1)],
                        start=(ci == 0), stop=(ci == 15),
                    )
                    if kind == "q" and h == 0:
                        _warm(4 if ci >= 8 else 2)  # fill DMA-pacing gaps
                    elif kind == "q" and h == 1:
                        _warm(2)
                    elif kind == "k" and h == 0:
                        _warm(1)
                if kind == "q":
                    nc.vector.tensor_copy(
                        qT_tiles[0][:, 512 * h:512 * (h + 1)], ps[:])
                else:
                    nc.vector.tensor_copy(kT_sb[:, T * h:T * h + 512], ps[:])

            # deferred so the first x chunk + weights win the DMA queue;
            # x1 feeds phase-0's proj(c1) filler units, wo phase-1's WO(0).
            x_tiles[1] = xs.tile([128, 8192], DT, tag="x", name="x_sb")
            nc.sync.dma_start(x_tiles[1][:, 0:4096], xt[:, 8192:12288])
            nc.sync.dma_start(x_tiles[1][:, 4096:8192], xt[:, 12288:16384])
            nc.sync.dma_start(wo_sb[:], wot[:])

            # ---------- global stream state ----------
            copyq = []

            def _flush_copy(n=1):
                # Deferred ob-copy emission: by the time the copy reaches
                # the head of its FIFO engine queue, the WO matmuls it
                # waits on are already done, so it can't head-of-line
                # block the exps/adds queued behind it.
                for _ in range(min(n, len(copyq))):
                    ps, obt, t0, cc, sc, fine = copyq.pop(0)
                    ob = obt[:, 512 * cc:512 * (cc + 1)]
                    if sc:
                        nc.scalar.copy(ob, ps[:])
                    else:
                        nc.vector.tensor_copy(ob, ps[:])
                    if fine:
                        nc.sync.dma_start(
                            out[t0:t0 + 128, 512 * cc:512 * (cc + 1)],
                            obt[:, 512 * cc:512 * (cc + 1)],
                        )
                    elif cc == 3:
                        nc.sync.dma_start(out[t0:t0 + 128, :], obt[:])

            obt_cur = [None]

            def _wo_group(wo_tci, ts, cc, scalar_copy=False, ptag="acc",
                          fine_dma=False):
                t0 = 512 * wo_tci + 128 * ts
                ps = psum.tile([128, 512], F32, tag=ptag,
                               bufs={"acc": 2, "o": 2, "l": 1}[ptag])
                for h in range(NH):
                    nc.tensor.matmul(
                        ps[:],
                        lhsT=oT_sb[:, T * h + t0:T * h + t0 + 128],
                        rhs=wo_sb[:, 2048 * h + 512 * cc:2048 * h + 512 * (cc + 1)],
                        start=(h == 0), stop=(h == NH - 1),
                    )
                if cc == 0:
                    obt_cur[0] = work.tile([128, 2048], DT_OUT, tag="ob",
                                           bufs=3, name="obt")
                copyq.append((ps, obt_cur[0], t0, cc, scalar_copy, fine_dma))

            def _proj_chain(kind, c1, hh):
                # one full 16-matmul projection chain for chunk c1
                x_sb = x_tiles[c1]
                ps = psum.tile([128, 512], F32, tag="acc", bufs=2)
                if kind == "v":
                    for ci in range(16):
                        nc.tensor.matmul(
                            ps[:],
                            lhsT=x_sb[:, 512 * ci + 128 * hh:512 * ci + 128 * (hh + 1)],
                            rhs=wv_sb[:, 512 * ci:512 * (ci + 1)],
                            start=(ci == 0), stop=(ci == 15),
                        )
                    tt = 4 * c1 + hh
                    nc.vector.tensor_copy(v_sb[:, 512 * tt:512 * (tt + 1)], ps[:])
                    return
                w_sb = wq_sb if kind == "q" else wk_sb
                for ci in range(16):
                    nc.tensor.matmul(
                        ps[:],
                        lhsT=w_sb[:, 2048 * hh + 128 * ci:2048 * hh + 128 * (ci + 1)],
                        rhs=x_sb[:, 512 * ci:512 * (ci + 1)],
                        start=(ci == 0), stop=(ci == 15),
                    )
                if kind == "q":
                    nc.vector.tensor_copy(
                        qT_tiles[c1][:, 512 * hh:512 * (hh + 1)], ps[:])
                else:
                    nc.vector.tensor_copy(
                        kT_sb[:, T * hh + 512 * c1:T * hh + 512 * (c1 + 1)], ps[:])

            # ---------- the four attention phases ----------
            for tci in range(NT):
                qT = qT_tiles[tci]
                nk_off = 4 * tci
                niter = NH * (nk_off + 4)

                # prefetch next x chunk + allocate next qT at phase start
                # (x1 is already DMA'd from the startup queue)
                if tci + 1 < NT:
                    if x_tiles[tci + 1] is None:
                        x_tiles[tci + 1] = xs.tile(
                            [128, 8192], DT, tag="x", name="x_sb")
                        nc.sync.dma_start(
                            x_tiles[tci + 1][:, 0:4096],
                            xt[:, 8192 * (tci + 1):8192 * (tci + 1) + 4096])
                        nc.sync.dma_start(
                            x_tiles[tci + 1][:, 4096:8192],
                            xt[:, 8192 * (tci + 1) + 4096:8192 * (tci + 2)])
                    qT_tiles[tci + 1] = work.tile([128, NH * 512], DT, tag="qT",
                                                  name="qT")

                # build this phase's filler unit list (each unit ~2048 PE
                # cycles): WO groups of the previous chunk first (ready
                # immediately, covers the x-DMA window), then next-chunk
                # projection chains.  Chunk3's V chains spill into phase 3.
                units = []
                if tci > 0:
                    groups = [(gi, ts, cc) for gi, (ts, cc) in enumerate(
                        (ts, cc) for ts in range(4) for cc in range(4))]
                    if tci == 2:
                        groups = groups[:12]   # last 4 spill into phase 3
                    for gi, ts, cc in groups:
                        units.append((lambda ts=ts, cc=cc, gi=gi, c0=tci - 1:
                                      _wo_group(c0, ts, cc,
                                                scalar_copy=(gi % 2 == 0))))
                    if tci == 3:
                        for gi, ts, cc in [(gi, ts, cc) for gi, (ts, cc)
                                           in enumerate((ts, cc)
                                           for ts in range(4)
                                           for cc in range(4))][12:]:
                            units.append(
                                (lambda ts=ts, cc=cc, gi=gi:
                                 _wo_group(1, ts, cc,
                                           scalar_copy=(gi % 2 == 0))))
                if tci + 1 < NT:
                    c1 = tci + 1
                    for hh in range(NH):
                        units.append(lambda hh=hh, c1=c1: _proj_chain("q", c1, hh))
                        units.append(lambda hh=hh, c1=c1: _proj_chain("k", c1, hh))
                    if c1 < NT - 1:
                        for ts in range(4):
                            units.append(
                                lambda ts=ts, c1=c1: _proj_chain("v", c1, ts))
                if tci == NT - 1:
                    # chunk3 V chains, deferred from phase 2: needed by
                    # PV(kt>=12), which arrives ~1/4 into each head's items
                    for ts in range(4):
                        units.insert(ts, (lambda ts=ts: _proj_chain("v", NT - 1, ts)))
                if tci == 0:
                    # chunk0's K(h2,h3) + V chains, deferred from the upfront
                    # section: K h2/h3 are needed by items 8+/12+, V by the
                    # first PV (pend-delayed), and their wk/wv DMAs land
                    # mid-phase, so they fill what used to be DMA stalls.
                    pre = ([lambda h=h: _proj_chain("k", 0, h) for h in (2, 3)]
                           + [lambda ts=ts: _proj_chain("v", 0, ts)
                              for ts in range(4)])
                    units = pre + units

                o_ps = {}
                l_ps = {}
                prev_p = {}
                pair1_of = {}
                stash = {}
                pend = []

                def _s_tile(w):
                    return psum.tile([128, w], F32, tag="s", bufs=3, name="s_ps")

                def _s_exp(h, kt):
                    if kt < nk_off:
                        # full-width off-diagonal tile
                        s_ps = _s_tile(512)
                        nc.tensor.matmul(
                            s_ps[:],
                            lhsT=kT_sb[:, T * h + 128 * kt:T * h + 128 * (kt + 1)],
                            rhs=qT[:, 512 * h:512 * (h + 1)],
                            start=True, stop=True,
                        )
                        p = work.tile([128, 512], DT, tag="p", bufs=6)
                        nc.scalar.activation(
                            p[:], s_ps[:], mybir.ActivationFunctionType.Exp,
                            scale=SCALE,
                        )
                        # pair/quad-sum on DVE; quads accumulate into the
                        # stash so ONE ones-matmul per (h, chunk) suffices
                        if kt % 2 == 0:
                            prev_p[h] = p
                        else:
                            pp = work.tile([128, 512], DT, tag="pp", bufs=3)
                            nc.vector.tensor_add(pp[:], prev_p[h][:], p[:])
                            if kt % 4 == 1:
                                pair1_of[h] = pp
                            else:
                                qq = work.tile([128, 512], DT, tag="qq", bufs=4)
                                nc.vector.tensor_add(qq[:], pair1_of.pop(h)[:], pp[:])
                                if h in stash:
                                    oo = work.tile([128, 512], DT, tag="qq",
                                                   bufs=4, name="oo")
                                    nc.vector.tensor_add(
                                        oo[:], stash.pop(h)[:], qq[:])
                                    stash[h] = oo
                                else:
                                    stash[h] = qq
                        return p
                    # diagonal-band tile m: q-width 512-128m, masked after exp
                    m = kt - nk_off
                    w = 512 - 128 * m
                    s_ps = _s_tile(w)
                    nc.tensor.matmul(
                        s_ps[:],
                        lhsT=kT_sb[:, T * h + 128 * kt:T * h + 128 * (kt + 1)],
                        rhs=qT[:, 512 * h + 128 * m:512 * (h + 1)],
                        start=True, stop=True,
                    )
                    p = work.tile([128, w], DT, tag="p", bufs=6, name="p")
                    nc.scalar.activation(
                        p[:], s_ps[:], mybir.ActivationFunctionType.Exp,
                        scale=SCALE,
                    )
                    nc.vector.tensor_mul(
                        p[:], p[:], msk_sb[:, 512 * m + 128 * m:512 * (m + 1)]
                    )
                    # fold the band into the stash tile (DVE), so the whole
                    # denominator is ONE 512-wide ones-matmul at the end.
                    # The stash is always a fresh accumulator tile: the later
                    # band adds must never mutate p itself, which the PV
                    # matmul still has to read.
                    if m == 0:
                        st = work.tile([128, 512], DT, tag="qq",
                                       bufs=4, name="oo")
                        if h in stash:
                            nc.vector.tensor_add(st[:], stash.pop(h)[:], p[:])
                        else:
                            nc.vector.tensor_copy(st[:], p[:])
                        stash[h] = st
                    else:
                        st = stash[h]
                        nc.vector.tensor_add(
                            st[:, 128 * m:512], st[:, 128 * m:512], p[:])
                    return p

                def _l_pv(h, kt, p):
                    if kt < nk_off:
                        nc.tensor.matmul(
                            o_ps[h][:],
                            lhsT=v_sb[:, 512 * kt + 128 * h:512 * kt + 128 * (h + 1)],
                            rhs=p[:],
                            start=(kt == 0), stop=False,
                        )
                    else:
                        m = kt - nk_off
                        nc.tensor.matmul(
                            o_ps[h][:, 128 * m:512],
                            lhsT=v_sb[:, 512 * kt + 128 * h:512 * kt + 128 * (h + 1)],
                            rhs=p[:],
                            start=(kt == 0), stop=(m == 3),
                        )

                def _epilogue(h, sliced=False):
                    nc.tensor.matmul(
                        l_ps[h][:], lhsT=ones_k[:], rhs=stash.pop(h)[:],
                        start=True, stop=True,
                    )
                    halves = 2 if sliced else 1
                    wq_ = 512 // halves
                    for half in range(halves):
                        sl = slice(wq_ * half, wq_ * (half + 1))
                        r_sb = work.tile([1, wq_], F32, tag="r")
                        nc.vector.reciprocal_approx_fast(r_sb[:], l_ps[h][:, sl])
                        rb_sb = work.tile([128, wq_], F32, tag="rb")
                        nc.gpsimd.partition_broadcast(rb_sb[:], r_sb[:])
                        nc.vector.tensor_mul(
                            oT_sb[:, T * h + 512 * tci + sl.start:
                                  T * h + 512 * tci + sl.stop],
                            o_ps[h][:, sl], rb_sb[:],
                        )

                # ---- Bresenham-interleaved stream: items + filler units ----
                nunit = len(units)
                ucred = 0.0
                uidx = 0
                for idx, (h, kt) in enumerate(
                    (h, kt) for h in range(NH) for kt in range(nk_off + 4)
                ):
                    if kt == 0:
                        o_ps[h] = psum.tile([128, 512], F32, tag="o", name="o_ps")
                        l_ps[h] = psum.tile([1, 512], F32, tag="l", name="l_ps",
                                            bufs=1)
                    pend.append((h, kt, _s_exp(h, kt)))
                    if len(pend) > 5:
                        ch, ckt, cp = pend.pop(0)
                        _l_pv(ch, ckt, cp)
                        if ckt == nk_off + 3:
                            _epilogue(ch)
                    if len(copyq) >= 2:
                        _flush_copy(1)
                    ucred += nunit / niter
                    while uidx < nunit and ucred >= 1.0:
                        units[uidx]()
                        uidx += 1
                        ucred -= 1.0
                for ch, ckt, cp in pend:
                    _l_pv(ch, ckt, cp)
                    if ckt == nk_off + 3:
                        _epilogue(ch, sliced=(tci == NT - 1 and ch == NH - 1))
                    if uidx < nunit:
                        units[uidx]()
                        uidx += 1
                        _flush_copy(1)
                while uidx < nunit:
                    units[uidx]()
                    uidx += 1
                    _flush_copy(1)
                pend = []

            # final chunk's output projection (tail): rotate through 5 PSUM
            # banks (acc/o/l tags are all free here) and both copy engines;
            # fine_dma streams each 512-col block out as soon as it's copied
            tail_tags = ["acc", "o", "acc", "o", "l"]
            for i, (ts, cc) in enumerate(
                (ts, cc) for ts in range(4) for cc in range(4)
            ):
                _wo_group(NT - 1, ts, cc, scalar_copy=(i % 2 == 0),
                          ptag=tail_tags[i % 5], fine_dma=True)
                _flush_copy(1)
            _flush_copy(99)
    nc.compile()
    return nc


_NC = None


def _get_nc():
    global _NC
    if _NC is None:
        _NC = _build()
    return _NC


def _pack_w(w, hg):
    # wv shard for head-group hg, pre-transposed + ci-tiled (moving operand):
    # out[p, 512*ci + d] = w[512*hg + d, 128*ci + p]
    wt = np.ascontiguousarray(w[512 * hg:512 * (hg + 1), :].T)  # [C, 512]
    return np.ascontiguousarray(
        wt.reshape(16, 128, 512).transpose(1, 0, 2).reshape(128, 8192)
    )


def _pack_w_hm(w, hg):
    # wq/wk shard, head-major so head h's stationary tiles are contiguous:
    # out[p, 2048*h + 128*ci + j] = w[512*hg + 128*h + j, 128*ci + p]
    wt = np.ascontiguousarray(w[512 * hg:512 * (hg + 1), :].T)  # [C, 512]
    return np.ascontiguousarray(
        wt.reshape(16, 128, 4, 128).transpose(1, 2, 0, 3).reshape(128, 8192)
    )


def _pack_wo(wo, hg):
    # wo columns for head-group hg, transposed + tiled by head:
    # out[p, 2048*h + c] = wo[c, 512*hg + 128*h + p]
    wt = np.ascontiguousarray(wo[:, 512 * hg:512 * (hg + 1)].T)  # [512, C]
    return np.ascontiguousarray(
        wt.reshape(4, 128, 2048).transpose(1, 0, 2).reshape(128, 8192)
    )


def _pack_x(xb):
    # x[b] transposed + tiled: out[p, 8192*tc + 512*ci + tt] = x[512*tc+tt, 128*ci+p]
    xT = np.ascontiguousarray(xb.T)  # [C, T]
    return np.ascontiguousarray(
        xT.reshape(16, 128, 4, 512).transpose(1, 2, 0, 3).reshape(128, 16 * T)
    )


def _diag_masks():
    kk = np.arange(128)[:, None]
    qq = np.arange(512)[None, :]
    blocks = [(128 * m + kk <= qq).astype(np.float32) for m in range(4)]
    return np.concatenate(blocks, axis=1)  # [128, 2048]


def _in_maps(x, wq, wk, wv, wo):
    msk = _diag_masks().astype(NP_BF16)
    xts = [_pack_x(x[b]).astype(NP_BF16) for b in range(B)]
    wqts = [_pack_w_hm(wq, g).astype(NP_BF16) for g in range(HG)]
    wkts = [_pack_w_hm(wk, g).astype(NP_BF16) for g in range(HG)]
    wvts = [_pack_w(wv, g).astype(NP_BF16) for g in range(HG)]
    wots = [_pack_wo(wo, g).astype(NP_BF16) for g in range(HG)]
    maps = []
    for i in range(8):
        b, g = divmod(i, HG)
        maps.append({
            "xt": xts[b], "wqt": wqts[g], "wkt": wkts[g], "wvt": wvts[g],
            "wot": wots[g], "msk": msk,
        })
    return maps


def _run(x, wq, wk, wv, wo, trace=False):
    nc = _get_nc()
    maps = _in_maps(x, wq, wk, wv, wo)
    res = run_bass_kernel_spmd(nc, maps, core_ids=list(range(8)), trace=trace)
    full = np.empty((B, T, C), dtype=np.float32)
    for b in range(B):
        acc = res.results[HG * b]["out"].astype(np.float32)
        for g in range(1, HG):
            acc = acc + res.results[HG * b + g]["out"].astype(np.float32)
        full[b] = acc
    return full, res


def kernel(x, mask=None, wq=None, wk=None, wv=None, wo=None, **_ignored):
    x = np.asarray(x, dtype=np.float32)
    wq = np.asarray(wq, dtype=np.float32)
    wk = np.asarray(wk, dtype=np.float32)
    wv = np.asarray(wv, dtype=np.float32)
    wo = np.asarray(wo, dtype=np.float32)
    full, _ = _run(x, wq, wk, wv, wo, trace=False)
    return full


# revision 14
# speedup vs baseline: 1.1963x; 1.1963x over previous
"""Causal multi-head attention on 8 TRN2 NeuronCores — v2 (phase-interleaved).

Problem: B=2, T=2048, C=2048, H=16 heads, D=128 head_dim, fp32 reference.

Sharding (hardcoded): tensor-parallel over heads x4 (4 heads per core),
data-parallel over batch x2.  Core i handles batch i//4, head-group i%4
(heads 4*(i%4) .. 4*(i%4)+3).  Each core computes a *partial* output
[T, C] = (softmax(QK^T/sqrt(D)) V)_heads @ wo_shard^T ; the host sums the
4 TP partials per batch (the row-parallel wo all-reduce, done at unshard).

v2 structure: the attention stream for q-chunk c is scalar-engine (exp)
bound, so the PE bubbles there are packed with independent filler work:
the NEXT chunk's Q/K/V projection chains and the PREVIOUS chunk's WO
groups, interleaved Bresenham-style between attention items.  The
denominator is folded to ONE ones-matmul per (head, chunk) by
accumulating all band partials into the quad-stash tile on the DVE.
"""

import math

import ml_dtypes
import numpy as np

import concourse.bass as bass
import concourse.tile as tile
from concourse import bacc, mybir
from concourse.bass_utils import run_bass_kernel_spmd

B, T, C = 2, 2048, 2048
H, D = 16, 128
HG = 4              # head-groups (TP degree); heads per core = H // HG = 4
NH = H // HG        # heads per core
NT = T // 512       # 512-wide t/q chunks
SCALE = 1.0 / math.sqrt(D)

BF16 = mybir.dt.bfloat16
F32 = mybir.dt.float32

NP_BF16 = ml_dtypes.bfloat16

DT = BF16           # all matmul operands bf16; PSUM accumulation is f32
DT_OUT = BF16       # partial-output DMA dtype (host sums partials in f32)


def _build():
    nc = bacc.Bacc("TRN2", target_bir_lowering=False, debug=False, num_devices=8)

    xt = nc.dram_tensor("xt", [128, 16 * T], DT, kind="ExternalInput")
    wqt = nc.dram_tensor("wqt", [128, 8192], DT, kind="ExternalInput")
    wkt = nc.dram_tensor("wkt", [128, 8192], DT, kind="ExternalInput")
    wvt = nc.dram_tensor("wvt", [128, 8192], DT, kind="ExternalInput")
    wot = nc.dram_tensor("wot", [128, 8192], DT, kind="ExternalInput")
    msk = nc.dram_tensor("msk", [128, 2048], DT, kind="ExternalInput")
    out = nc.dram_tensor("out", [T, C], DT_OUT, kind="ExternalOutput")

    with tile.TileContext(nc) as tc:
        with (
            tc.tile_pool(name="big", bufs=1) as big,
            tc.tile_pool(name="xs", bufs=2) as xs,
            tc.tile_pool(name="work", bufs=2) as work,
            tc.tile_pool(name="ps", bufs=2, space="PSUM") as psum,
        ):
            wq_sb = big.tile([128, 8192], DT)
            wk_sb = big.tile([128, 8192], DT)
            wv_sb = big.tile([128, 8192], DT)
            wo_sb = big.tile([128, 8192], DT)
            msk_sb = big.tile([128, 2048], DT)
            ones_k = big.tile([128, 1], DT)
            warm_sb = big.tile([128, 128], DT)
            nc.gpsimd.memset(ones_k[:], 1.0)
            nc.gpsimd.memset(warm_sb[:], 1.0)

            # Startup DMAs on BOTH hardware DGE queues (Sync + Scalar run
            # independent rings): Sync carries the Q-path (wq, x0, x1),
            # Scalar carries the K/V-path (wk, wv) plus msk/wo, so the
            # DMA-bound startup window is nearly halved.
            x_tiles = [None] * NT
            x_tiles[0] = xs.tile([128, 8192], DT, tag="x", name="x_sb")
            x0_sb = x_tiles[0]
            nc.sync.dma_start(wq_sb[:, 0:2048], wqt[:, 0:2048])
            nc.sync.dma_start(x0_sb[:, 0:2048], xt[:, 0:2048])
            nc.sync.dma_start(x0_sb[:, 2048:4096], xt[:, 2048:4096])
            nc.sync.dma_start(x0_sb[:, 4096:6144], xt[:, 4096:6144])
            nc.sync.dma_start(x0_sb[:, 6144:8192], xt[:, 6144:8192])
            nc.sync.dma_start(wq_sb[:, 2048:4096], wqt[:, 2048:4096])
            nc.sync.dma_start(wk_sb[:, 0:2048], wkt[:, 0:2048])
            nc.sync.dma_start(wk_sb[:, 2048:4096], wkt[:, 2048:4096])
            nc.sync.dma_start(wq_sb[:, 4096:8192], wqt[:, 4096:8192])
            nc.sync.dma_start(msk_sb[:], msk[:])
            nc.sync.dma_start(wk_sb[:, 4096:8192], wkt[:, 4096:8192])
            nc.sync.dma_start(wv_sb[:], wvt[:])

            # Dummy matmul stream during the initial DMA wait: keeps the PE
            # busy through the HAM activity window so the real matmuls start
            # at the un-throttled clock.  More warm matmuls are interleaved
            # into the DMA-paced first Q projection below.
            warm_ps = psum.tile([1, 128], F32, tag="s", name="warm_ps", bufs=3)

            def _warm(n):
                for _ in range(n):
                    nc.tensor.matmul(
                        warm_ps[:], lhsT=ones_k[:], rhs=warm_sb[:],
                        start=True, stop=True,
                    )

            _warm(32)

            kT_sb = big.tile([128, NH * T], DT)     # per head: [d=128, t]
            v_sb = big.tile([128, 16 * 512], DT)    # [t=128, (t_tile, 4h*128)]
            oT_sb = big.tile([128, NH * T], DT)     # per head: [d=128, t]

            qT_tiles = [None] * NT

            # ---- chunk-0 Q + K(h0,h1) projections, DMA-paced with warm
            # fill.  K(h2,h3) and V0 become phase-0 filler units, so the PE
            # does real work instead of stalling on the late wk/wv DMAs.
            qT_tiles[0] = work.tile([128, NH * 512], DT, tag="qT", name="qT")
            for h, kind in [(0, "q"), (1, "q"), (0, "k"), (1, "k"),
                            (2, "q"), (3, "q")]:
                w_sb0 = wq_sb if kind == "q" else wk_sb
                ps = psum.tile([128, 512], F32, tag="acc", bufs=2)
                for ci in range(16):
                    nc.tensor.matmul(
                        ps[:],
                        lhsT=w_sb0[:, 2048 * h + 128 * ci:2048 * h + 128 * (ci + 1)],
                        rhs=x0_sb[:, 512 * ci:512 * (ci + 1)],
                        start=(ci == 0), stop=(ci == 15),
                    )
                    if kind == "q" and h == 0:
                        _warm(4 if ci >= 8 else 2)  # fill DMA-pacing gaps
                    elif kind == "q" and h == 1:
                        _warm(2)
                    elif kind == "k" and h == 0:
                        _warm(1)
                if kind == "q":
                    nc.vector.tensor_copy(
                        qT_tiles[0][:, 512 * h:512 * (h + 1)], ps[:])
                else:
                    nc.vector.tensor_copy(kT_sb[:, T * h:T * h + 512], ps[:])

            # deferred so the first x chunk + weights win the DMA queue;
            # x1 feeds phase-0's proj(c1) filler units, wo phase-1's WO(0).
            x_tiles[1] = xs.tile([128, 8192], DT, tag="x", name="x_sb")
            nc.sync.dma_start(x_tiles[1][:, 0:4096], xt[:, 8192:12288])
            nc.sync.dma_start(x_tiles[1][:, 4096:8192], xt[:, 12288:16384])
            nc.sync.dma_start(wo_sb[:], wot[:])

            # ---------- global stream state ----------
            copyq = []

            def _flush_copy(n=1):
                # Deferred ob-copy emission: by the time the copy reaches
                # the head of its FIFO engine queue, the WO matmuls it
                # waits on are already done, so it can't head-of-line
                # block the exps/adds queued behind it.
                for _ in range(min(n, len(copyq))):
                    ps, obt, t0, cc, sc, fine = copyq.pop(0)
                    ob = obt[:, 512 * cc:512 * (cc + 1)]
                    if sc:
                        nc.scalar.copy(ob, ps[:])
                    else:
                        nc.vector.tensor_copy(ob, ps[:])
                    if fine:
                        nc.sync.dma_start(
                            out[t0:t0 + 128, 512 * cc:512 * (cc + 1)],
                            obt[:, 512 * cc:512 * (cc + 1)],
                        )
                    elif cc == 3:
                        nc.sync.dma_start(out[t0:t0 + 128, :], obt[:])

            obt_cur = [None]

            def _wo_group(wo_tci, ts, cc, scalar_copy=False, ptag="acc",
                          fine_dma=False):
                t0 = 512 * wo_tci + 128 * ts
                ps = psum.tile([128, 512], F32, tag=ptag,
                               bufs={"acc": 2, "o": 2, "l": 1}[ptag])
                for h in range(NH):
                    nc.tensor.matmul(
                        ps[:],
                        lhsT=oT_sb[:, T * h + t0:T * h + t0 + 128],
                        rhs=wo_sb[:, 2048 * h + 512 * cc:2048 * h + 512 * (cc + 1)],
                        start=(h == 0), stop=(h == NH - 1),
                    )
                if cc == 0:
                    obt_cur[0] = work.tile([128, 2048], DT_OUT, tag="ob",
                                           bufs=3, name="obt")
                copyq.append((ps, obt_cur[0], t0, cc, scalar_copy, fine_dma))

            def _proj_chain(kind, c1, hh):
                # one full 16-matmul projection chain for chunk c1
                x_sb = x_tiles[c1]
                ps = psum.tile([128, 512], F32, tag="acc", bufs=2)
                if kind == "v":
                    for ci in range(16):
                        nc.tensor.matmul(
                            ps[:],
                            lhsT=x_sb[:, 512 * ci + 128 * hh:512 * ci + 128 * (hh + 1)],
                            rhs=wv_sb[:, 512 * ci:512 * (ci + 1)],
                            start=(ci == 0), stop=(ci == 15),
                        )
                    tt = 4 * c1 + hh
                    nc.vector.tensor_copy(v_sb[:, 512 * tt:512 * (tt + 1)], ps[:])
                    return
                w_sb = wq_sb if kind == "q" else wk_sb
                for ci in range(16):
                    nc.tensor.matmul(
                        ps[:],
                        lhsT=w_sb[:, 2048 * hh + 128 * ci:2048 * hh + 128 * (ci + 1)],
                        rhs=x_sb[:, 512 * ci:512 * (ci + 1)],
                        start=(ci == 0), stop=(ci == 15),
                    )
                if kind == "q":
                    nc.vector.tensor_copy(
                        qT_tiles[c1][:, 512 * hh:512 * (hh + 1)], ps[:])
                else:
                    nc.vector.tensor_copy(
                        kT_sb[:, T * hh + 512 * c1:T * hh + 512 * (c1 + 1)], ps[:])

            # ---------- the four attention phases ----------
            for tci in range(NT):
                qT = qT_tiles[tci]
                nk_off = 4 * tci
                niter = NH * (nk_off + 4)

                # prefetch next x chunk + allocate next qT at phase start
                # (x1 is already DMA'd from the startup queue)
                if tci + 1 < NT:
                    if x_tiles[tci + 1] is None:
                        x_tiles[tci + 1] = xs.tile(
                            [128, 8192], DT, tag="x", name="x_sb")
                        nc.sync.dma_start(
                            x_tiles[tci + 1][:, 0:4096],
                            xt[:, 8192 * (tci + 1):8192 * (tci + 1) + 4096])
                        nc.sync.dma_start(
                            x_tiles[tci + 1][:, 4096:8192],
                            xt[:, 8192 * (tci + 1) + 4096:8192 * (tci + 2)])
                    qT_tiles[tci + 1] = work.tile([128, NH * 512], DT, tag="qT",
                                                  name="qT")

                # build this phase's filler unit list (each unit ~2048 PE
                # cycles): WO groups of the previous chunk first (ready
                # immediately, covers the x-DMA window), then next-chunk
                # projection chains.  Chunk3's V chains spill into phase 3.
                units = []
                if tci > 0:
                    groups = [(gi, ts, cc) for gi, (ts, cc) in enumerate(
                        (ts, cc) for ts in range(4) for cc in range(4))]
                    if tci == 2:
                        groups = groups[:12]   # last 4 spill into phase 3
                    for gi, ts, cc in groups:
                        units.append((lambda ts=ts, cc=cc, gi=gi, c0=tci - 1:
                                      _wo_group(c0, ts, cc,
                                                scalar_copy=(gi % 2 == 0))))
                    if tci == 3:
                        for gi, ts, cc in [(gi, ts, cc) for gi, (ts, cc)
                                           in enumerate((ts, cc)
                                           for ts in range(4)
                                           for cc in range(4))][12:]:
                            units.append(
                                (lambda ts=ts, cc=cc, gi=gi:
                                 _wo_group(1, ts, cc,
                                           scalar_copy=(gi % 2 == 0))))
                if tci + 1 < NT:
                    c1 = tci + 1
                    for hh in range(NH):
                        units.append(lambda hh=hh, c1=c1: _proj_chain("q", c1, hh))
                        units.append(lambda hh=hh, c1=c1: _proj_chain("k", c1, hh))
                    if c1 < NT - 1:
                        for ts in range(4):
                            units.append(
                                lambda ts=ts, c1=c1: _proj_chain("v", c1, ts))
                if tci == NT - 1:
                    # chunk3 V chains, deferred from phase 2: needed by
                    # PV(kt>=12), which arrives ~1/4 into each head's items
                    for ts in range(4):
                        units.insert(ts, (lambda ts=ts: _proj_chain("v", NT - 1, ts)))
                if tci == 0:
                    # chunk0's K(h2,h3) + V chains, deferred from the upfront
                    # section: K h2/h3 are needed by items 8+/12+, V by the
                    # first PV (pend-delayed), and their wk/wv DMAs land
                    # mid-phase, so they fill what used to be DMA stalls.
                    pre = ([lambda h=h: _proj_chain("k", 0, h) for h in (2, 3)]
                           + [lambda ts=ts: _proj_chain("v", 0, ts)
                              for ts in range(4)])
                    units = pre + units

                o_ps = {}
                l_ps = {}
                prev_p = {}
                pair1_of = {}
                stash = {}
                pend = []

                def _s_tile(w):
                    return psum.tile([128, w], F32, tag="s", bufs=3, name="s_ps")

                def _s_exp(h, kt):
                    if kt < nk_off:
                        # full-width off-diagonal tile
                        s_ps = _s_tile(512)
                        nc.tensor.matmul(
                            s_ps[:],
                            lhsT=kT_sb[:, T * h + 128 * kt:T * h + 128 * (kt + 1)],
                            rhs=qT[:, 512 * h:512 * (h + 1)],
                            start=True, stop=True,
                        )
                        p = work.tile([128, 512], DT, tag="p", bufs=6)
                        nc.scalar.activation(
                            p[:], s_ps[:], mybir.ActivationFunctionType.Exp,
                            scale=SCALE,
                        )
                        # pair/quad-sum on DVE; quads accumulate into the
                        # stash so ONE ones-matmul per (h, chunk) suffices
                        if kt % 2 == 0:
                            prev_p[h] = p
                        else:
                            pp = work.tile([128, 512], DT, tag="pp", bufs=3)
                            nc.vector.tensor_add(pp[:], prev_p[h][:], p[:])
                            if kt % 4 == 1:
                                pair1_of[h] = pp
                            else:
                                qq = work.tile([128, 512], DT, tag="qq", bufs=4)
                                nc.vector.tensor_add(qq[:], pair1_of.pop(h)[:], pp[:])
                                if h in stash:
                                    oo = work.tile([128, 512], DT, tag="qq",
                                                   bufs=4, name="oo")
                                    nc.vector.tensor_add(
                                        oo[:], stash.pop(h)[:], qq[:])
                                    stash[h] = oo
                                else:
                                    stash[h] = qq
                        return p
                    # diagonal-band tile m: q-width 512-128m, masked after exp
                    m = kt - nk_off
                    w = 512 - 128 * m
                    s_ps = _s_tile(w)
                    nc.tensor.matmul(
                        s_ps[:],
                        lhsT=kT_sb[:, T * h + 128 * kt:T * h + 128 * (kt + 1)],
                        rhs=qT[:, 512 * h + 128 * m:512 * (h + 1)],
                        start=True, stop=True,
                    )
                    p = work.tile([128, w], DT, tag="p", bufs=6, name="p")
                    nc.scalar.activation(
                        p[:], s_ps[:], mybir.ActivationFunctionType.Exp,
                        scale=SCALE,
                    )
                    nc.vector.tensor_mul(
                        p[:], p[:], msk_sb[:, 512 * m + 128 * m:512 * (m + 1)]
                    )
                    # fold the band into the stash tile (DVE), so the whole
                    # denominator is ONE 512-wide ones-matmul at the end.
                    # The stash is always a fresh accumulator tile: the later
                    # band adds must never mutate p itself, which the PV
                    # matmul still has to read.
                    if m == 0:
                        st = work.tile([128, 512], DT, tag="qq",
                                       bufs=4, name="oo")
                        if h in stash:
                            nc.vector.tensor_add(st[:], stash.pop(h)[:], p[:])
                        else:
                            nc.vector.tensor_copy(st[:], p[:])
                        stash[h] = st
                    else:
                        st = stash[h]
                        nc.vector.tensor_add(
                            st[:, 128 * m:512], st[:, 128 * m:512], p[:])
                    return p

                def _l_pv(h, kt, p):
                    if kt < nk_off:
                        nc.tensor.matmul(
                            o_ps[h][:],
                            lhsT=v_sb[:, 512 * kt + 128 * h:512 * kt + 128 * (h + 1)],
                            rhs=p[:],
                            start=(kt == 0), stop=False,
                        )
                    else:
                        m = kt - nk_off
                        nc.tensor.matmul(
                            o_ps[h][:, 128 * m:512],
                            lhsT=v_sb[:, 512 * kt + 128 * h:512 * kt + 128 * (h + 1)],
                            rhs=p[:],
                            start=(kt == 0), stop=(m == 3),
                        )

                def _epilogue(h, sliced=False):
                    nc.tensor.matmul(
                        l_ps[h][:], lhsT=ones_k[:], rhs=stash.pop(h)[:],
                        start=True, stop=True,
                    )
                    halves = 2 if sliced else 1
                    wq_ = 512 // halves
                    for half in range(halves):
                        sl = slice(wq_ * half, wq_ * (half + 1))
                        r_sb = work.tile([1, wq_], F32, tag="r")
                        nc.vector.reciprocal_approx_fast(r_sb[:], l_ps[h][:, sl])
                        rb_sb = work.tile([128, wq_], F32, tag="rb")
                        nc.gpsimd.partition_broadcast(rb_sb[:], r_sb[:])
                        nc.vector.tensor_mul(
                            oT_sb[:, T * h + 512 * tci + sl.start:
                                  T * h + 512 * tci + sl.stop],
                            o_ps[h][:, sl], rb_sb[:],
                        )

                # ---- Bresenham-interleaved stream: items + filler units ----
                nunit = len(units)
                ucred = 0.0
                uidx = 0
                for idx, (h, kt) in enumerate(
                    (h, kt) for h in range(NH) for kt in range(nk_off + 4)
                ):
                    if kt == 0:
                        o_ps[h] = psum.tile([128, 512], F32, tag="o", name="o_ps")
                        l_ps[h] = psum.tile([1, 512], F32, tag="l", name="l_ps",
                                            bufs=1)
                    pend.append((h, kt, _s_exp(h, kt)))
                    if len(pend) > 5:
                        ch, ckt, cp = pend.pop(0)
                        _l_pv(ch, ckt, cp)
                        if ckt == nk_off + 3:
                            _epilogue(ch)
                    if len(copyq) >= 2:
                        _flush_copy(1)
                    ucred += nunit / niter
                    while uidx < nunit and ucred >= 1.0:
                        units[uidx]()
                        uidx += 1
                        ucred -= 1.0
                for ch, ckt, cp in pend:
                    _l_pv(ch, ckt, cp)
                    if ckt == nk_off + 3:
                        _epilogue(ch, sliced=(tci == NT - 1 and ch == NH - 1))
                while uidx < nunit:
                    units[uidx]()
                    uidx += 1
                    _flush_copy(1)
                pend = []

            # final chunk's output projection (tail): rotate through 5 PSUM
            # banks (acc/o/l tags are all free here) and both copy engines;
            # fine_dma streams each 512-col block out as soon as it's copied
            tail_tags = ["acc", "o", "acc", "o", "l"]
            for i, (ts, cc) in enumerate(
                (ts, cc) for ts in range(4) for cc in range(4)
            ):
                _wo_group(NT - 1, ts, cc, scalar_copy=(i % 2 == 0),
                          ptag=tail_tags[i % 5], fine_dma=True)
                _flush_copy(1)
            _flush_copy(99)
    nc.compile()
    return nc


_NC = None


def _get_nc():
    global _NC
    if _NC is None:
        _NC = _build()
    return _NC


def _pack_w(w, hg):
    # wv shard for head-group hg, pre-transposed + ci-tiled (moving operand):
    # out[p, 512*ci + d] = w[512*hg + d, 128*ci + p]
    wt = np.ascontiguousarray(w[512 * hg:512 * (hg + 1), :].T)  # [C, 512]
    return np.ascontiguousarray(
        wt.reshape(16, 128, 512).transpose(1, 0, 2).reshape(128, 8192)
    )


def _pack_w_hm(w, hg):
    # wq/wk shard, head-major so head h's stationary tiles are contiguous:
    # out[p, 2048*h + 128*ci + j] = w[512*hg + 128*h + j, 128*ci + p]
    wt = np.ascontiguousarray(w[512 * hg:512 * (hg + 1), :].T)  # [C, 512]
    return np.ascontiguousarray(
        wt.reshape(16, 128, 4, 128).transpose(1, 2, 0, 3).reshape(128, 8192)
    )


def _pack_wo(wo, hg):
    # wo columns for head-group hg, transposed + tiled by head:
    # out[p, 2048*h + c] = wo[c, 512*hg + 128*h + p]
    wt = np.ascontiguousarray(wo[:, 512 * hg:512 * (hg + 1)].T)  # [512, C]
    return np.ascontiguousarray(
        wt.reshape(4, 128, 2048).transpose(1, 0, 2).reshape(128, 8192)
    )


def _pack_x(xb):
    # x[b] transposed + tiled: out[p, 8192*tc + 512*ci + tt] = x[512*tc+tt, 128*ci+p]
    xT = np.ascontiguousarray(xb.T)  # [C, T]
    return np.ascontiguousarray(
        xT.reshape(16, 128, 4, 512).transpose(1, 2, 0, 3).reshape(128, 16 * T)
    )


def _diag_masks():
    kk = np.arange(128)[:, None]
    qq = np.arange(512)[None, :]
    blocks = [(128 * m + kk <= qq).astype(np.float32) for m in range(4)]
    return np.concatenate(blocks, axis=1)  # [128, 2048]


def _in_maps(x, wq, wk, wv, wo):
    msk = _diag_masks().astype(NP_BF16)
    xts = [_pack_x(x[b]).astype(NP_BF16) for b in range(B)]
    wqts = [_pack_w_hm(wq, g).astype(NP_BF16) for g in range(HG)]
    wkts = [_pack_w_hm(wk, g).astype(NP_BF16) for g in range(HG)]
    wvts = [_pack_w(wv, g).astype(NP_BF16) for g in range(HG)]
    wots = [_pack_wo(wo, g).astype(NP_BF16) for g in range(HG)]
    maps = []
    for i in range(8):
        b, g = divmod(i, HG)
        maps.append({
            "xt": xts[b], "wqt": wqts[g], "wkt": wkts[g], "wvt": wvts[g],
            "wot": wots[g], "msk": msk,
        })
    return maps


def _run(x, wq, wk, wv, wo, trace=False):
    nc = _get_nc()
    maps = _in_maps(x, wq, wk, wv, wo)
    res = run_bass_kernel_spmd(nc, maps, core_ids=list(range(8)), trace=trace)
    full = np.empty((B, T, C), dtype=np.float32)
    for b in range(B):
        acc = res.results[HG * b]["out"].astype(np.float32)
        for g in range(1, HG):
            acc = acc + res.results[HG * b + g]["out"].astype(np.float32)
        full[b] = acc
    return full, res


def kernel(x, mask=None, wq=None, wk=None, wv=None, wo=None, **_ignored):
    x = np.asarray(x, dtype=np.float32)
    wq = np.asarray(wq, dtype=np.float32)
    wk = np.asarray(wk, dtype=np.float32)
    wv = np.asarray(wv, dtype=np.float32)
    wo = np.asarray(wo, dtype=np.float32)
    full, _ = _run(x, wq, wk, wv, wo, trace=False)
    return full
